# revision 13
# baseline (speedup 1.0000x reference)
"""Trainium2 Bass kernel for nn_PoolWithHole: 3x3 max-pool excluding the
center tap, zero-padded borders, clamped at 0 (torch running-max-from-zeros):

    out[b,i,j] = max(0, max_{(di,dj)!=(0,0), |di|<=1, |dj|<=1} x[b,i+di,j+dj])

Sharding: pure data parallel over batch B=64 -> 8 NeuronCores x 8 images.
Exact fp32 (bit-identical to the reference; absmax err == 0).

Per-core layout: image rows -> SBUF partitions, cols -> free dim.
TRN2 engine ops must start at partition 0/32/64/96, so vertical
(partition-axis) shifts cannot be expressed as shifted operands of a
DVE op.  They are produced instead by TensorE permutation matmuls into
PSUM (exact: 0/1 weights, one product per output; fp32 splits
recombine exactly for a single-term dot product).

Per 126-output-row tile (input rows o0-1 .. o0+126 at partitions 0..127):
    DVE  P[k]   = max(x[2k], x[2k+1])         pairwise (w/2+1 wide)
    DVE  h[2k]  = max(P[k],  x[2k+2])         | decimated 3-tap row max:
    DVE  h[2k+1]= max(P[k+1],x[2k+1])         | 1.5 ops/elem instead of 2
    DVE  m2[j]  = max(x[j-1], x[j+1])         row pair with hole
    PE   S2h[p] = h[p+2]      (shift-by-2 matmul -> PSUM)
    PE   S1m[p] = m2[p+1]     (shift-by-1 matmul -> PSUM)
    ACT  S2c    = copy(S2h), S1c = relu(S1m)  (PSUM -> SBUF evacuation;
                  relu folds the zero clamp at zero DVE cost)
    DVE  w      = max(h, S2c)                 = max(h above, h below)
    DVE  out    = max(w, S1c)
Zero padding: zeroed pad columns (memset) + zeroed halo rows at the
image top/bottom (tiny DMA from a zeros input); the final max(...,0)
makes the extra zero candidates harmless.

The kernel is DVE-bound (97% busy in the device-occupancy timeline):
4.5 fp32 tensor_tensor-class ops/element at 1x (0.96 GHz, 1
elem/lane/cyc) ~= 381 us/core predicted (TimelineSim; an equivalent HW
chained-execution measurement agreed within 5%), vs the ~187 us HBM
roofline (64 MB/core @ ~358 GB/s).  Binary max has no other home on
TRN2: ScalarE is unary (it contributes the PSUM evacuation + relu
above), TensorE is linear (it contributes the two shifts), and this
toolchain's walrus rejects all compute opcodes on GpSimd.
"""

import os
import sys

sys.path.insert(0, "/opt/trn_rl_repo")
os.environ.setdefault("MYCRO_LOCAL_CACHE", "1")

import numpy as np
from contextlib import ExitStack

import concourse.bass as bass  # noqa: F401  (registers AP machinery)
from concourse import bacc, mybir
import concourse.tile as tile
from concourse import bass_utils

F32 = mybir.dt.float32
MAX = mybir.AluOpType.max
RELU = mybir.ActivationFunctionType.Relu
COPY = mybir.ActivationFunctionType.Copy

# AP class for hand-built access patterns (fused interleaved h op)
_APC = None


def _ap_class():
    global _APC
    if _APC is None:
        _APC = type(
            bass.Bass("TRN2", target_bir_lowering=False)
            .alloc_sbuf_tensor("_apq", [1, 1], F32)
            .ap()
        )
    return _APC


def _mkap(base, doffset, dims):
    """Arbitrary affine AP into base's tensor: dims = [[step, count], ...]."""
    return _ap_class()(base.tensor, base.offset + doffset, dims)

N_CORES = 8
FULL_B, H, W = 64, 1024, 1024
B_LOCAL = FULL_B // N_CORES

_NC_CACHE: dict = {}


def shift_matrices() -> np.ndarray:
    """lhsT pair [128, 252]: cols 0:126 shift-by-2, cols 126:252 shift-by-1.

    out = lhsT.T @ rhs, so lhsT[k, p] = 1 picks rhs[k] into out[p]."""
    m = np.zeros((128, 252), dtype=np.float32)
    for p in range(126):
        m[p + 2, p] = 1.0
        m[p + 1, 126 + p] = 1.0
    return m


def build_nc(b_local: int, h: int, w: int):
    nc = bacc.Bacc(
        "TRN2",
        target_bir_lowering=False,
        debug=False,
        enable_asserts=False,
        num_devices=N_CORES,
    )
    x = nc.dram_tensor("x", [b_local, h, w], F32, kind="ExternalInput").ap()
    shm = nc.dram_tensor("shm", [128, 252], F32, kind="ExternalInput").ap()
    zrow = nc.dram_tensor("zrow", [1, w + 2], F32, kind="ExternalInput").ap()
    out = nc.dram_tensor("out", [b_local, h, w], F32, kind="ExternalOutput").ap()

    TO = 126
    ntiles = (h + TO - 1) // TO
    NCHUNK = 512  # fp32 matmul moving-operand / PSUM-bank limit

    with tile.TileContext(nc) as tc, ExitStack() as ctx:
        cp = ctx.enter_context(tc.tile_pool(name="const", bufs=1))
        xp = ctx.enter_context(tc.tile_pool(name="xp", bufs=4))
        tp = ctx.enter_context(tc.tile_pool(name="tp", bufs=3))
        hp = ctx.enter_context(tc.tile_pool(name="hp", bufs=3))
        mp = ctx.enter_context(tc.tile_pool(name="mp", bufs=3))
        wp_ = ctx.enter_context(tc.tile_pool(name="wp", bufs=3))
        op_ = ctx.enter_context(tc.tile_pool(name="op", bufs=4))
        pp = ctx.enter_context(tc.tile_pool(name="psum", bufs=2, space="PSUM"))
        sp = ctx.enter_context(tc.tile_pool(name="spp", bufs=2, space="PSUM"))
        s2c_p = ctx.enter_context(tc.tile_pool(name="s2c", bufs=3))
        s1c_p = ctx.enter_context(tc.tile_pool(name="s1c", bufs=3))

        SH = cp.tile([128, 252], F32)
        nc.sync.dma_start(SH[:, :], shm[:, :])

        # Persistent X buffers: pad columns zeroed once, never overwritten
        # (the per-tile DMA writes only cols 1..w), so no per-tile memsets
        # grabbing the shared DVE/GpSimd SBUF port.
        XB = 4
        xbufs = []
        for i in range(XB):
            Xi = xp.tile([128, w + 2], F32, tag=f"Xb{i}")
            nc.gpsimd.memset(Xi[:, 0:1], 0.0)
            nc.gpsimd.memset(Xi[:, w + 1 : w + 2], 0.0)
            xbufs.append(Xi)
        it = 0

        for b in range(b_local):
            for t in range(ntiles):
                o0 = t * TO
                n_out = min(TO, h - o0)
                p_cnt = n_out + 2  # input rows spanned (incl halo)
                r_lo, r_hi = o0 - 1, o0 + n_out
                lo_clip, hi_clip = r_lo < 0, r_hi > h - 1
                lr_lo, lr_hi = max(r_lo, 0), min(r_hi, h - 1)
                nrows = lr_hi - lr_lo + 1
                p0 = 1 if lo_clip else 0  # partition of first loaded row

                X = xbufs[it % XB]
                it += 1
                if lo_clip:
                    nc.sync.dma_start(X[0:1, :], zrow[:, :])
                if hi_clip:
                    nc.sync.dma_start(X[p_cnt - 1 : p_cnt, :], zrow[:, :])
                nc.sync.dma_start(
                    X[p0 : p0 + nrows, 1 : w + 1], x[b, lr_lo : lr_hi + 1, :]
                )

                # Decimated 3-tap row max (1.5 ops/elem instead of 2):
                #   P[k]    = max(X[2k], X[2k+1])             (w/2+1 wide)
                #   h[2k]   = max(P[k],   X[2k+2])            (even cols)
                #   h[2k+1] = max(P[k+1], X[2k+1])            (odd cols)
                hw2 = w // 2
                P = tp.tile([128, hw2 + 1], F32)
                nc.vector.tensor_max(
                    P[0:p_cnt, :], X[0:p_cnt, 0 : w + 1 : 2],
                    X[0:p_cnt, 1 : w + 2 : 2],
                )
                # Both strided h combines in one instruction via a 3D AP:
                #   s=0: h[2k]   = max(P[k],   X[2k+2])
                #   s=1: h[2k+1] = max(P[k+1], X[2k+1])
                # out s-step +1, P s-step +1, X s-step -1 (from col 2).
                Hh = hp.tile([128, w], F32)
                Hb, Pb, Xb = Hh[:, :], P[:, :], X[:, :]
                nc.vector.tensor_tensor(
                    _mkap(Hb, 0, [[Hb.ap[0][0], p_cnt], [1, 2], [2, hw2]]),
                    _mkap(Pb, 0, [[Pb.ap[0][0], p_cnt], [1, 2], [1, hw2]]),
                    _mkap(Xb, 2, [[Xb.ap[0][0], p_cnt], [-1, 2], [2, hw2]]),
                    MAX,
                )
                M2 = mp.tile([128, w], F32)
                nc.vector.tensor_max(
                    M2[0:p_cnt, :], X[0:p_cnt, 0:w], X[0:p_cnt, 2 : w + 2]
                )

                S2h = pp.tile([126, w], F32)
                for c0 in range(0, w, NCHUNK):
                    c1 = min(c0 + NCHUNK, w)
                    nc.tensor.matmul(
                        S2h[:, c0:c1], SH[0:p_cnt, 0:126], Hh[0:p_cnt, c0:c1]
                    )
                S1m = sp.tile([126, w], F32)
                for c0 in range(0, w, NCHUNK):
                    c1 = min(c0 + NCHUNK, w)
                    nc.tensor.matmul(
                        S1m[:, c0:c1], SH[0:p_cnt, 126:252], M2[0:p_cnt, c0:c1]
                    )

                # ScalarE (own SBUF/PSUM ports, otherwise idle) evacuates the
                # PSUM shift results so the DVE combines run SBUF-only (58- vs
                # 120-cycle init), and folds the max(...,0) into a free Relu.
                S2c = s2c_p.tile([126, w], F32)
                nc.scalar.activation(S2c[0:n_out, :], S2h[0:n_out, :], COPY)
                S1c = s1c_p.tile([126, w], F32)
                nc.scalar.activation(S1c[0:n_out, :], S1m[0:n_out, :], RELU)
                Wt = wp_.tile([126, w], F32)
                nc.vector.tensor_max(
                    Wt[0:n_out, :], Hh[0:n_out, :], S2c[0:n_out, :]
                )
                O = op_.tile([126, w], F32)
                nc.vector.tensor_max(
                    O[0:n_out, :], Wt[0:n_out, :], S1c[0:n_out, :]
                )
                nc.sync.dma_start(out[b, o0 : o0 + n_out, :], O[0:n_out, :])

    nc.compile()
    return nc


def _get_nc(b_local: int, h: int, w: int):
    key = (b_local, h, w)
    if key not in _NC_CACHE:
        _NC_CACHE[key] = build_nc(b_local, h, w)
    return _NC_CACHE[key]


def _in_maps(x: np.ndarray, b_local: int, w: int):
    shm = shift_matrices()
    zrow = np.zeros((1, w + 2), dtype=np.float32)
    return [
        {
            "x": np.ascontiguousarray(x[i * b_local : (i + 1) * b_local]),
            "shm": shm,
            "zrow": zrow,
        }
        for i in range(N_CORES)
    ]


def kernel(x: np.ndarray, **_unused) -> np.ndarray:
    """Full-input entry point: x [64,1024,1024] fp32 -> out same shape."""
    x = np.asarray(x)
    assert x.shape == (FULL_B, H, W), x.shape
    nc = _get_nc(B_LOCAL, H, W)
    res = bass_utils.run_bass_kernel_spmd(
        nc, _in_maps(x, B_LOCAL, W), core_ids=list(range(N_CORES))
    )
    return np.concatenate([r["out"] for r in res.results], axis=0)
